# revision 1
# baseline (speedup 1.0000x reference)
"""Bass/Tile Mamba layer kernel for TRN2, 8 cores.

Sharding: core k -> batch b = k//4, d_inner shard s = k%4 (512 channels).
All activations on device live in [channel_partition, time_free] layout; the
host pre-transposes x and all weights so no on-device transposes are needed.

Per-core pipeline over t-blocks (TB columns each):
  in-proj (fp32r matmuls) -> causal conv (gpsimd) + silu -> x_proj partial ->
  AllReduce(x_dbl) over the 4 shard cores -> dt = ln(1+exp(raw+bias)) ->
  per-(cblock, state-pair): a = exp(A[c,n]*dt) (ACT), b = B_n*u (gpsimd),
  h = tensor_tensor_scan(a, b) (DVE), w = C_n*h (DVE),
  y += w via gpsimd DMA-accumulate into DRAM tiles ->
  y = (y + D*x_conv)*silu(z) -> out-proj (bf16) -> partial out [o, t] to HBM.
Host sums the 4 shard partials per batch and transposes back to [B, T, D].
"""
import numpy as np
import ml_dtypes

import concourse.bass as bass
import concourse.mybir as mybir
from concourse import bacc
from concourse.tile import TileContext

F32 = mybir.dt.float32
F32R = mybir.dt.float32r
BF16 = mybir.dt.bfloat16

D_MODEL = 1024
D_STATE = 16
D_CONV = 4
D_INNER = 2048
DT_RANK = 64
B_SZ = 2
SEQ = 4096
NCORES = 8
CSH = D_INNER // 4          # channels per core = 512
NCB = CSH // 128            # c-blocks per core = 4
KM = D_MODEL // 128         # k-tiles over d_model = 8
NOB = D_MODEL // 128        # out blocks = 8

REPLICA_GROUPS = [[0, 1, 2, 3], [4, 5, 6, 7]]
AF = mybir.ActivationFunctionType
OP = mybir.AluOpType


def build(nc, T=SEQ, TB=512):
    NBLK = T // TB
    NPAIR = D_STATE // 2

    xT = nc.dram_tensor("xT", [D_MODEL, T], F32R, kind="ExternalInput").ap()
    winT = nc.dram_tensor("winT", [D_MODEL, 2 * CSH], F32R, kind="ExternalInput").ap()
    wxT = nc.dram_tensor("wxT", [CSH, 96], F32R, kind="ExternalInput").ap()
    wdtT = nc.dram_tensor("wdtT", [DT_RANK, CSH], F32R, kind="ExternalInput").ap()
    woutT = nc.dram_tensor("woutT", [CSH, D_MODEL], BF16, kind="ExternalInput").ap()
    # [128, NCB, 23]: 0..3 conv_w, 4 conv_b, 5 b_dt, 6 D, 7..22 A[c, n]
    consts = nc.dram_tensor("consts", [128, NCB, 23], F32, kind="ExternalInput").ap()
    selr = nc.dram_tensor("selr", [D_STATE, D_STATE * 128], F32R, kind="ExternalInput").ap()
    out_part = nc.dram_tensor("out_part", [D_MODEL, T], F32, kind="ExternalOutput").ap()

    ccin = nc.dram_tensor("ccin", [NBLK, 96, TB], F32, kind="Internal").ap()
    ccout = nc.dram_tensor("ccout", [NBLK, 96, TB], F32, kind="Internal").ap()

    with TileContext(nc) as tc:
        with tc.tile_pool(name="wpool", bufs=1) as wpool, \
             tc.tile_pool(name="xin", bufs=1) as xin, \
             tc.tile_pool(name="blk", bufs=2) as blk, \
             tc.tile_pool(name="scanp", bufs=2) as scanp, \
             tc.tile_pool(name="psA", bufs=3, space="PSUM") as psA, \
             tc.tile_pool(name="psB", bufs=1, space="PSUM") as psB, \
             tc.tile_pool(name="ydram", bufs=2, space="DRAM") as ydram:

            # ---- weights / constants (resident) ----
            win_t = [wpool.tile([128, 2 * CSH], F32R, tag=f"win{j}", name=f"win{j}") for j in range(KM)]
            for j in range(KM):
                nc.sync.dma_start(win_t[j][:], winT[128 * j:128 * (j + 1), :])
            wx_t = [wpool.tile([128, 96], F32R, tag=f"wx{c}", name=f"wx{c}") for c in range(NCB)]
            for c in range(NCB):
                nc.sync.dma_start(wx_t[c][:], wxT[128 * c:128 * (c + 1), :])
            wdt_t = wpool.tile([DT_RANK, CSH], F32R, tag="wdt")
            nc.sync.dma_start(wdt_t[:], wdtT)
            wout_t = [wpool.tile([128, D_MODEL], BF16, tag=f"wo{c}", name=f"wo{c}") for c in range(NCB)]
            for c in range(NCB):
                nc.sync.dma_start(wout_t[c][:], woutT[128 * c:128 * (c + 1), :])
            cst = wpool.tile([128, NCB, 23], F32, tag="cst")
            nc.sync.dma_start(cst[:], consts)
            sel_t = wpool.tile([D_STATE, D_STATE, 128], F32R, tag="sel")
            nc.sync.dma_start(sel_t[:], selr.rearrange("k (n p) -> k n p", p=128))

            hcar = wpool.tile([128, NCB, D_STATE], F32, tag="hcar")
            nc.vector.memset(hcar[:], 0.0)
            xi_prev = wpool.tile([128, NCB, D_CONV - 1], F32, tag="xiprev")
            nc.vector.memset(xi_prev[:], 0.0)

            for k in range(NBLK):
                ts = slice(k * TB, (k + 1) * TB)

                xt = xin.tile([128, KM, TB], F32R, tag="xt")
                for j in range(KM):
                    nc.sync.dma_start(xt[:, j, :], xT[128 * j:128 * (j + 1), ts])

                # ---- in-proj x-part, conv, silu ----
                xc = []
                for cb in range(NCB):
                    ps = psA.tile([128, TB], F32, tag="ps")
                    for j in range(KM):
                        nc.tensor.matmul(ps[:], win_t[j][:, 128 * cb:128 * (cb + 1)],
                                         xt[:, j, :], start=(j == 0), stop=(j == KM - 1))
                    t_xi = blk.tile([128, TB + D_CONV - 1], F32, tag="xi")
                    nc.scalar.copy(t_xi[:, D_CONV - 1:], ps[:])
                    nc.vector.tensor_copy(t_xi[:, 0:D_CONV - 1], xi_prev[:, cb, :])
                    nc.vector.tensor_copy(xi_prev[:, cb, :], t_xi[:, TB:TB + D_CONV - 1])
                    q0 = blk.tile([128, TB], F32, tag="cq0")
                    q1 = blk.tile([128, TB], F32, tag="cq1")
                    nc.gpsimd.tensor_scalar(q0[:], t_xi[:, 0:TB], cst[:, cb, 0:1],
                                            cst[:, cb, 4:5], OP.mult, OP.add)
                    nc.gpsimd.tensor_scalar(q1[:], t_xi[:, 1:1 + TB], cst[:, cb, 1:2],
                                            None, OP.mult)
                    nc.gpsimd.tensor_tensor(q0[:], q0[:], q1[:], OP.add)
                    nc.gpsimd.tensor_scalar(q1[:], t_xi[:, 2:2 + TB], cst[:, cb, 2:3],
                                            None, OP.mult)
                    nc.gpsimd.tensor_tensor(q0[:], q0[:], q1[:], OP.add)
                    nc.gpsimd.tensor_scalar(q1[:], t_xi[:, 3:3 + TB], cst[:, cb, 3:4],
                                            None, OP.mult)
                    nc.gpsimd.tensor_tensor(q0[:], q0[:], q1[:], OP.add)
                    sgc = blk.tile([128, TB], F32, tag="sgc")
                    nc.scalar.activation(sgc[:], q0[:], AF.Sigmoid)
                    t_xc = blk.tile([128, TB], F32R, tag=f"xc{cb}")
                    nc.gpsimd.tensor_tensor(t_xc[:], q0[:], sgc[:], OP.mult)
                    xc.append(t_xc)

                # ---- in-proj z-part, silu(z), spill to DRAM ----
                zdr = []
                for cb in range(NCB):
                    ps = psA.tile([128, TB], F32, tag="ps")
                    for j in range(KM):
                        nc.tensor.matmul(ps[:], win_t[j][:, CSH + 128 * cb:CSH + 128 * (cb + 1)],
                                         xt[:, j, :], start=(j == 0), stop=(j == KM - 1))
                    sg = blk.tile([128, TB], F32, tag="sg")
                    nc.scalar.activation(sg[:], ps[:], AF.Sigmoid)
                    t_zs = blk.tile([128, TB], F32, tag="zs")
                    nc.vector.tensor_tensor(t_zs[:], sg[:], ps[:], OP.mult)
                    t_zdr = ydram.tile([128, TB], F32, tag=f"zdr{cb}")
                    nc.sync.dma_start(t_zdr[:], t_zs[:])
                    zdr.append(t_zdr)

                # ---- x_dbl partial -> allreduce -> dt_low/B/C ----
                psx = psB.tile([96, TB], F32, tag="bps")
                for cb in range(NCB):
                    nc.tensor.matmul(psx[:], wx_t[cb][:], xc[cb][:],
                                     start=(cb == 0), stop=(cb == NCB - 1))
                sxs = blk.tile([96, TB], F32, tag="sxs")
                nc.scalar.copy(sxs[:], psx[:])
                nc.sync.dma_start(ccin[k], sxs[:])
                nc.gpsimd.collective_compute(
                    "AllReduce", OP.add, replica_groups=REPLICA_GROUPS,
                    ins=[ccin[k]], outs=[ccout[k]])
                dtl = blk.tile([DT_RANK, TB], F32R, tag="dtl")
                nc.sync.dma_start(dtl[:], ccout[k, 0:DT_RANK, :].bitcast(F32R))
                brow = blk.tile([D_STATE, TB], F32R, tag="brow")
                nc.sync.dma_start(brow[:], ccout[k, DT_RANK:80, :].bitcast(F32R))
                crow = blk.tile([D_STATE, TB], F32R, tag="crow")
                nc.sync.dma_start(crow[:], ccout[k, 80:96, :].bitcast(F32R))

                # ---- dt, u, y-init ----
                dt_t, u_t, ydr = [], [], []
                for cb in range(NCB):
                    psd = psA.tile([128, TB], F32, tag="ps")
                    nc.tensor.matmul(psd[:], wdt_t[:, 128 * cb:128 * (cb + 1)], dtl[:],
                                     start=True, stop=True)
                    e = blk.tile([128, TB], F32, tag="edt")
                    nc.scalar.activation(e[:], psd[:], AF.Exp, bias=cst[:, cb, 5:6])
                    t_dt = blk.tile([128, TB], F32, tag=f"dt{cb}")
                    nc.scalar.activation(t_dt[:], e[:], AF.Ln, bias=1.0)
                    dt_t.append(t_dt)
                    t_u = blk.tile([128, TB], BF16, tag=f"u{cb}")
                    nc.gpsimd.tensor_tensor(t_u[:], t_dt[:], xc[cb][:].bitcast(F32),
                                            OP.mult)
                    u_t.append(t_u)
                    t_yd = blk.tile([128, TB], F32, tag="yd")
                    nc.gpsimd.tensor_scalar(t_yd[:], xc[cb][:].bitcast(F32),
                                            cst[:, cb, 6:7], None, OP.mult)
                    t_ydr = ydram.tile([128, TB], F32, tag=f"ydr{cb}")
                    nc.sync.dma_start(t_ydr[:], t_yd[:])
                    ydr.append(t_ydr)

                # ---- scan phase: pairs of states ----
                for pr in range(NPAIR):
                    n0 = 2 * pr
                    bps = psB.tile([128, 2, TB], F32, tag="bps")
                    for i in range(2):
                        nc.tensor.matmul(bps[:, i, :], sel_t[:, n0 + i, :],
                                         brow[:], start=True, stop=True)
                    bbs = scanp.tile([128, 2, TB], BF16, tag="bbs")
                    nc.vector.tensor_copy(bbs[:], bps[:])
                    cps = psB.tile([128, 2, TB], F32, tag="cps")
                    for i in range(2):
                        nc.tensor.matmul(cps[:, i, :], sel_t[:, n0 + i, :],
                                         crow[:], start=True, stop=True)
                    cbs = scanp.tile([128, 2, TB], BF16, tag="cbs")
                    nc.vector.tensor_copy(cbs[:], cps[:])

                    for cb in range(NCB):
                        a2 = scanp.tile([128, 2, TB], F32, tag="a2")
                        for i in range(2):
                            nc.scalar.activation(
                                a2[:, i, :], dt_t[cb][:], AF.Exp,
                                scale=cst[:, cb, 7 + n0 + i:8 + n0 + i])
                        b2 = scanp.tile([128, 2, TB], BF16, tag="b2")
                        nc.gpsimd.tensor_tensor(
                            b2[:], bbs[:],
                            u_t[cb][:, None, :].to_broadcast([128, 2, TB]),
                            OP.mult)
                        h2 = scanp.tile([128, 2, TB], BF16, tag="h2")
                        for i in range(2):
                            nc.vector.tensor_tensor_scan(
                                h2[:, i, :], a2[:, i, :], b2[:, i, :],
                                hcar[:, cb, n0 + i:n0 + i + 1], OP.mult, OP.add)
                        nc.vector.tensor_copy(hcar[:, cb, n0:n0 + 2], h2[:, :, TB - 1])
                        w2 = scanp.tile([128, 2, TB], BF16, tag="w2")
                        nc.vector.tensor_tensor(w2[:], h2[:], cbs[:], OP.mult)
                        for i in range(2):
                            nc.gpsimd.dma_start(ydr[cb][:], w2[:, i, :],
                                                accum_op=OP.add)

                # ---- z-gate + out-proj ----
                yz = []
                for cb in range(NCB):
                    t_y = blk.tile([128, TB], F32, tag="yrb")
                    nc.sync.dma_start(t_y[:], ydr[cb][:])
                    t_zs = blk.tile([128, TB], F32, tag="zrb")
                    nc.sync.dma_start(t_zs[:], zdr[cb][:])
                    t_yz = blk.tile([128, TB], BF16, tag=f"yz{cb}")
                    nc.vector.tensor_tensor(t_yz[:], t_y[:], t_zs[:], OP.mult)
                    yz.append(t_yz)
                for ob in range(NOB):
                    pso = psA.tile([128, TB], F32, tag="ps")
                    for cb in range(NCB):
                        nc.tensor.matmul(pso[:], wout_t[cb][:, 128 * ob:128 * (ob + 1)],
                                         yz[cb][:], start=(cb == 0), stop=(cb == NCB - 1))
                    so = blk.tile([128, TB], F32, tag="so")
                    nc.scalar.copy(so[:], pso[:])
                    nc.sync.dma_start(out_part[128 * ob:128 * (ob + 1), ts], so[:])
    return nc


def build_module(T=SEQ, TB=512):
    nc = bacc.Bacc("TRN2", target_bir_lowering=False, debug=False, num_devices=NCORES)
    build(nc, T=T, TB=TB)
    nc.compile()
    return nc


def make_core_inputs(x, W_in, conv_w, conv_b, W_x, W_dt, b_dt, A_log, D, W_out,
                     core, T=SEQ):
    b = core // 4
    s = core % 4
    c0, c1 = CSH * s, CSH * (s + 1)
    xTc = np.ascontiguousarray(np.asarray(x)[b, :T, :].T).astype(np.float32)
    winTc = np.ascontiguousarray(
        np.concatenate([np.asarray(W_in)[c0:c1, :],
                        np.asarray(W_in)[D_INNER + c0:D_INNER + c1, :]], axis=0).T
    ).astype(np.float32)
    wxTc = np.ascontiguousarray(np.asarray(W_x)[:, c0:c1].T).astype(np.float32)
    wdtTc = np.ascontiguousarray(np.asarray(W_dt)[c0:c1, :].T).astype(np.float32)
    woutTc = np.ascontiguousarray(np.asarray(W_out)[:, c0:c1].T).astype(ml_dtypes.bfloat16)
    consts = np.zeros((128, NCB, 23), np.float32)
    A = (-np.exp(np.asarray(A_log)[c0:c1, :].astype(np.float64))).astype(np.float32)
    cw = np.asarray(conv_w).reshape(D_INNER, D_CONV)
    for cb in range(NCB):
        rows = slice(128 * cb, 128 * (cb + 1))
        consts[:, cb, 0:4] = cw[c0:c1][rows]
        consts[:, cb, 4] = np.asarray(conv_b)[c0:c1][rows]
        consts[:, cb, 5] = np.asarray(b_dt)[c0:c1][rows]
        consts[:, cb, 6] = np.asarray(D)[c0:c1][rows]
        consts[:, cb, 7:23] = A[rows]
    selv = np.zeros((D_STATE, D_STATE, 128), np.float32)
    for n in range(D_STATE):
        selv[n, n, :] = 1.0
    selv = selv.reshape(D_STATE, D_STATE * 128)
    return {"xT": xTc, "winT": winTc, "wxT": wxTc, "wdtT": wdtTc,
            "woutT": woutTc, "consts": consts, "selr": selv}


def gather_output(results, T=SEQ):
    out = np.zeros((B_SZ, T, D_MODEL), np.float32)
    for b in range(B_SZ):
        acc = np.zeros((D_MODEL, T), np.float32)
        for s in range(4):
            acc += results[4 * b + s]["out_part"]
        out[b] = acc.T
    return out


# ----------------------------------------------------------------------------
# Harness entry point: kernel(**inputs) -> full output [2, 4096, 1024] fp32.
# Builds/compiles the Bass module once per process, shards inputs across the
# 8 NeuronCores (batch x d_inner-shard), runs SPMD, and gathers on host.
# ----------------------------------------------------------------------------
from concourse.bass_utils import run_bass_kernel_spmd

_NC_CACHE = {}


def _get_module():
    if "nc" not in _NC_CACHE:
        _NC_CACHE["nc"] = build_module(T=SEQ, TB=512)
    return _NC_CACHE["nc"]


def kernel(x, W_in, conv_w, conv_b, W_x, W_dt, b_dt, A_log, D, W_out):
    nc = _get_module()
    args = dict(x=x, W_in=W_in, conv_w=conv_w, conv_b=conv_b, W_x=W_x,
                W_dt=W_dt, b_dt=b_dt, A_log=A_log, D=D, W_out=W_out)
    in_maps = [make_core_inputs(**args, core=c, T=SEQ) for c in range(NCORES)]
    res = run_bass_kernel_spmd(nc, in_maps, core_ids=list(range(NCORES)))
    return gather_output(res.results, T=SEQ)
